# revision 38
# baseline (speedup 1.0000x reference)
"""CBOW negative-sampling loss kernel for 8 trn2 NeuronCores.

Strategy (data-parallel over batch):
  - Host concatenates W_target/W_context into one bf16 table [2V, D] and
    builds per-batch-element combined row indices [B, 17]
    (target, context+V, neg_0+V..neg_14+V).
  - Each core handles B/8 = 16384 batch elements, 128 tiles of 128.
  - Per tile: one indirect (gather) DMA pulls 17*128 rows of 256B from HBM
    into SBUF with batch on partitions; DVE computes
    emb_in = emb_t * mask, prods = emb_in * emb_j, tree-folds the 128-d
    segments, reduces to 16 scores; ACT computes ln(sigmoid(-x)) with a
    fused per-partition accumulation (= -softplus(x) summed over j).
  - Final: per-core [128,1] f32 partial sums -> host sum -> loss.
"""

import os

import numpy as np
import ml_dtypes

import concourse.bass as bass
import concourse.mybir as mybir
import concourse.tile as tile
from concourse import bacc, bass_utils

V, D, B, NEGS = 100000, 128, 131072, 15
NCORES = 8
BLOC = B // NCORES  # 16384
P = 128
T = BLOC // P  # 128 tiles per core
J = 2 + NEGS  # 17 gathered rows per batch element
G = 4  # tiles per gather call

BF16 = mybir.dt.bfloat16
F32 = mybir.dt.float32
NPBF16 = ml_dtypes.bfloat16

_CACHE = {}
LAST_RESULT = None  # BassKernelResults of the most recent run (for profiling)


def _get_dot_scan_op():
    """Register (once) a custom DVE op: out = running-sum of Src0*Src1 over
    the streamed free dims (fp32 state, downcast out). Segment sums are then
    strided differences of the stream at segment boundaries."""
    from concourse import dve_ops as D

    if "DOT_SCAN_ANT" in D._SUB_OPCODE_FOR_NAME:
        return _CACHE["dot_scan"]
    from concourse.dve_spec import AluOp, Spec, Src0, Src1, lower, scan
    from concourse.dve_uop import DveOpSpec

    def _ref(in0, in1, *_unused):
        p = in0.shape[0]
        a = in0.astype(np.float32).reshape(p, -1)
        b = np.asarray(in1).astype(np.float32).reshape(p, -1)
        if b.shape[1] != a.shape[1]:
            reps = a.shape[1] // b.shape[1]
            b = np.tile(b.reshape(p, 1, -1), (1, reps, 1)).reshape(p, -1)
        return np.cumsum(a * b, axis=-1).astype(in0.dtype).reshape(in0.shape)

    spec = Spec(body=scan(AluOp.ADD, Src0 * Src1), reference=_ref)
    row = max(D._SUB_OPCODE_FOR_NAME.values()) + 1
    shas = {}
    for ver in ("v3", "v4"):
        s = DveOpSpec(
            name="DOT_SCAN_ANT", opcode=row, uops=lower(spec, ver=ver), rd1_en=True
        )
        shas[ver] = s.sha(ver)
    op = D.DveOp("DOT_SCAN_ANT", spec, subdim=False, uops_sha=shas)
    D.OPS.append(op)
    D._SUB_OPCODE_FOR_NAME[op.name] = row
    D.CUSTOM_DVE_SPECS[op.name] = op.spec
    _CACHE["dot_scan"] = op
    return op


def _build_nc(V=V, T=T, G=G):
    nc = bacc.Bacc("TRN2", target_bir_lowering=False, debug=False)
    w = nc.dram_tensor("w_cat", [2 * V, D], BF16, kind="ExternalInput")
    idx = nc.dram_tensor("idx", [P, T * J], mybir.dt.int32, kind="ExternalInput")
    mask = nc.dram_tensor("maskr", [P, T * D], BF16, kind="ExternalInput")
    out = nc.dram_tensor("out", [P, 1], F32, kind="ExternalOutput")

    with tile.TileContext(nc) as tc:
        with (
            tc.tile_pool(name="const", bufs=1) as constp,
            tc.tile_pool(name="gather", bufs=5) as gatherp,
            tc.tile_pool(name="work", bufs=3) as workp,
            tc.tile_pool(name="small", bufs=4) as smallp,
        ):
            idx_sb = constp.tile([P, T * J], mybir.dt.int32)
            # first group's indices land first so gather 0 launches early
            nc.sync.dma_start(idx_sb[:, 0 : G * J], idx[:, 0 : G * J])
            nc.sync.dma_start(idx_sb[:, G * J :], idx[:, G * J :])
            mask_sb = constp.tile([P, T * D], BF16)
            # chunked so the first tiles' mask arrives quickly
            MCH = 16 if T >= 16 else 1
            for m in range(MCH):
                mc = T * D // MCH
                nc.sync.dma_start(
                    mask_sb[:, m * mc : (m + 1) * mc], mask[:, m * mc : (m + 1) * mc]
                )
            # ACT function tables: sigmoid and ln live in different table
            # sets (1283ns reload on switch), so run all sigmoids in the main
            # loop and one ln+accumulate pass at the end.
            LN_CHUNKS = min(8, T // G)
            GROUPS_PER_CHUNK = (T // G) // LN_CHUNKS
            CHUNK_COLS = G * 16 * GROUPS_PER_CHUNK
            tsum = constp.tile([P, LN_CHUNKS], F32)
            ln_scratch = constp.tile([P, CHUNK_COLS], F32)

            for g in range(T // G):
                emb = gatherp.tile([P, G * J * D], BF16, tag="emb")
                nc.gpsimd.indirect_dma_start(
                    out=emb[:],
                    out_offset=None,
                    in_=w[:],
                    in_offset=bass.IndirectOffsetOnAxis(
                        ap=idx_sb[:, g * G * J : (g + 1) * G * J], axis=0
                    ),
                )
                t0 = g * G
                # per-tile batch: emb_in for all G tiles in one op
                emb_in4 = smallp.tile([P, G * D], BF16, tag="embin")
                ei4 = emb_in4[:].rearrange("p (k d) -> p k d", d=D)
                nc.vector.tensor_tensor(
                    out=ei4,
                    in0=emb[:].rearrange("p (k j d) -> p k j d", j=J, d=D)[:, :, 0, :],
                    in1=mask_sb[:, t0 * D : (t0 + G) * D].rearrange(
                        "p (k d) -> p k d", d=D
                    ),
                    op=mybir.AluOpType.mult,
                )
                # fused custom DVE op: running sum of emb_j*emb_in across the
                # whole group stream; per-(tile,j) dots are then differences
                # at the 128-element segment boundaries.
                dot_scan = _get_dot_scan_op()
                scan4 = workp.tile([P, G * 16 * D], BF16, tag="prods")
                for k in range(G):
                    base = k * J * D
                    nc.vector._custom_dve(
                        dot_scan,
                        out=scan4[:, k * 16 * D : (k + 1) * 16 * D].rearrange(
                            "p (j d) -> p j d", d=D
                        ),
                        in0=emb[:, base + D : base + J * D].rearrange(
                            "p (j d) -> p j d", d=D
                        ),
                        in1=emb_in4[:, k * D : (k + 1) * D]
                        .unsqueeze(1)
                        .broadcast_to((P, 16, D)),
                    )
                S = G * 16
                bnd = scan4[:].rearrange("p (s d) -> p s d", d=D)[:, :, D - 1 : D]
                scores = smallp.tile([P, S], F32, tag="scores")
                # x[s] = cum[s] - cum[s-1] within each tile's scan; the scan
                # state resets per call, so tile-leading segments (s % 16 == 0)
                # take the raw boundary value instead (second op overwrites).
                nc.vector.tensor_tensor(
                    out=scores[:, 1:S].unsqueeze(2),
                    in0=bnd[:, 1:S, :],
                    in1=bnd[:, 0 : S - 1, :],
                    op=mybir.AluOpType.subtract,
                )
                nc.vector.tensor_copy(
                    scores[:, 0:S:16].unsqueeze(2), bnd[:, 0:S:16, :]
                )
                if g % GROUPS_PER_CHUNK == 0:
                    sig_c = smallp.tile([P, CHUNK_COLS], F32, tag="sigc")
                gc = g % GROUPS_PER_CHUNK
                nc.scalar.activation(
                    sig_c[:, gc * G * 16 : (gc + 1) * G * 16],
                    scores[:],
                    mybir.ActivationFunctionType.Sigmoid,
                    scale=-1.0,
                )
                # ln(sigmoid(-x)) = -softplus(x); accumulate per chunk,
                # interleaved so only the last chunk sits on the tail.
                if (g + 1) % GROUPS_PER_CHUNK == 0:
                    c = (g + 1) // GROUPS_PER_CHUNK - 1
                    nc.scalar.activation(
                        ln_scratch[:],
                        sig_c[:],
                        mybir.ActivationFunctionType.Ln,
                        accum_out=tsum[:, c : c + 1],
                    )

            total = constp.tile([P, 1], F32)
            nc.vector.tensor_reduce(
                total[:], tsum[:], axis=mybir.AxisListType.X, op=mybir.AluOpType.add
            )
            nc.sync.dma_start(out[:], total[:])
    nc.compile()
    return nc


def _get_nc():
    if "nc" not in _CACHE:
        _CACHE["nc"] = _build_nc()
    return _CACHE["nc"]


def kernel(target, context, neg_idx, dropout_mask, W_target, W_context):
    global LAST_RESULT
    nc = _get_nc()

    target = np.asarray(target).astype(np.int32, copy=False)
    context = np.asarray(context).astype(np.int32, copy=False)
    neg_idx = np.asarray(neg_idx).astype(np.int32, copy=False)
    dropout_mask = np.asarray(dropout_mask, dtype=np.float32)
    W_target = np.asarray(W_target, dtype=np.float32)
    W_context = np.asarray(W_context, dtype=np.float32)

    w_cat = np.ascontiguousarray(
        np.concatenate([W_target, W_context], axis=0).astype(NPBF16)
    )
    idx_cat = np.empty((B, J), np.int32)
    idx_cat[:, 0] = target
    idx_cat[:, 1] = context + V
    idx_cat[:, 2:] = neg_idx + V
    mask_bf = dropout_mask.astype(NPBF16)

    in_maps = []
    for c in range(NCORES):
        sl = slice(c * BLOC, (c + 1) * BLOC)
        idxs = np.ascontiguousarray(
            idx_cat[sl].reshape(T, P, J).transpose(1, 0, 2).reshape(P, T * J)
        )
        maskr = np.ascontiguousarray(
            mask_bf[sl].reshape(T, P, D).transpose(1, 0, 2).reshape(P, T * D)
        )
        in_maps.append({"w_cat": w_cat, "idx": idxs, "maskr": maskr})

    trace = bool(int(os.environ.get("KERNEL_TRACE", "0")))
    res = bass_utils.run_bass_kernel_spmd(
        nc, in_maps, core_ids=list(range(NCORES)), trace=trace
    )
    LAST_RESULT = res

    tot = 0.0
    for r in res.results:
        tot += float(r["out"].astype(np.float64).sum())
    # device accumulated sum of ln(sigmoid(-x)) = -sum of softplus(x)
    loss = -tot / B
    return np.asarray(np.float32(loss))


# revision 43
# speedup vs baseline: 1.1855x; 1.1855x over previous
"""CBOW negative-sampling loss kernel for 8 trn2 NeuronCores.

Strategy (data-parallel over batch):
  - Host concatenates W_target/W_context into one bf16 table [2V, D] and
    builds per-batch-element combined row indices [B, 17]
    (target, context+V, neg_0+V..neg_14+V).
  - Each core handles B/8 = 16384 batch elements, 128 tiles of 128.
  - Per tile: one indirect (gather) DMA pulls 17*128 rows of 256B from HBM
    into SBUF with batch on partitions; DVE computes
    emb_in = emb_t * mask, prods = emb_in * emb_j, tree-folds the 128-d
    segments, reduces to 16 scores; ACT computes ln(sigmoid(-x)) with a
    fused per-partition accumulation (= -softplus(x) summed over j).
  - Final: per-core [128,1] f32 partial sums -> host sum -> loss.
"""

import os

import numpy as np
import ml_dtypes

import concourse.bass as bass
import concourse.mybir as mybir
import concourse.tile as tile
from concourse import bacc, bass_utils

V, D, B, NEGS = 100000, 128, 131072, 15
NCORES = 8
BLOC = B // NCORES  # 16384
P = 128
T = BLOC // P  # 128 tiles per core
J = 2 + NEGS  # 17 gathered rows per batch element
G = 4  # tiles per gather call

BF16 = mybir.dt.bfloat16
F32 = mybir.dt.float32
NPBF16 = ml_dtypes.bfloat16

_CACHE = {}
LAST_RESULT = None  # BassKernelResults of the most recent run (for profiling)


def _get_dot_scan_op():
    """Register (once) a custom DVE op: out = running-sum of Src0*Src1 over
    the streamed free dims (fp32 state, downcast out). Segment sums are then
    strided differences of the stream at segment boundaries."""
    from concourse import dve_ops as D

    if "DOT_SCAN_ANT" in D._SUB_OPCODE_FOR_NAME:
        return _CACHE["dot_scan"]
    from concourse.dve_spec import AluOp, Spec, Src0, Src1, lower, scan
    from concourse.dve_uop import DveOpSpec

    def _ref(in0, in1, *_unused):
        p = in0.shape[0]
        a = in0.astype(np.float32).reshape(p, -1)
        b = np.asarray(in1).astype(np.float32).reshape(p, -1)
        if b.shape[1] != a.shape[1]:
            reps = a.shape[1] // b.shape[1]
            b = np.tile(b.reshape(p, 1, -1), (1, reps, 1)).reshape(p, -1)
        return np.cumsum(a * b, axis=-1).astype(in0.dtype).reshape(in0.shape)

    spec = Spec(body=scan(AluOp.ADD, Src0 * Src1), reference=_ref)
    row = max(D._SUB_OPCODE_FOR_NAME.values()) + 1
    shas = {}
    for ver in ("v3", "v4"):
        s = DveOpSpec(
            name="DOT_SCAN_ANT", opcode=row, uops=lower(spec, ver=ver), rd1_en=True
        )
        shas[ver] = s.sha(ver)
    op = D.DveOp("DOT_SCAN_ANT", spec, subdim=False, uops_sha=shas)
    D.OPS.append(op)
    D._SUB_OPCODE_FOR_NAME[op.name] = row
    D.CUSTOM_DVE_SPECS[op.name] = op.spec
    _CACHE["dot_scan"] = op
    return op


def _build_nc(V=V, T=T, G=G):
    nc = bacc.Bacc("TRN2", target_bir_lowering=False, debug=False)
    w = nc.dram_tensor("w_cat", [2 * V, D], BF16, kind="ExternalInput")
    idx = nc.dram_tensor("idx", [P, T * J], mybir.dt.int32, kind="ExternalInput")
    mask = nc.dram_tensor("maskr", [P, T * D], BF16, kind="ExternalInput")
    out = nc.dram_tensor("out", [P, 1], F32, kind="ExternalOutput")

    with tile.TileContext(nc) as tc:
        with (
            tc.tile_pool(name="const", bufs=1) as constp,
            tc.tile_pool(name="gather", bufs=5) as gatherp,
            tc.tile_pool(name="work", bufs=3) as workp,
            tc.tile_pool(name="small", bufs=3) as smallp,
        ):
            idx_sb = constp.tile([P, T * J], mybir.dt.int32)
            # first group's indices land first so gather 0 launches early
            nc.sync.dma_start(idx_sb[:, 0 : G * J], idx[:, 0 : G * J])
            nc.sync.dma_start(idx_sb[:, G * J :], idx[:, G * J :])
            mask_sb = constp.tile([P, T * D], BF16)
            # chunked so the first tiles' mask arrives quickly
            MCH = 16 if T >= 16 else 1
            for m in range(MCH):
                mc = T * D // MCH
                nc.sync.dma_start(
                    mask_sb[:, m * mc : (m + 1) * mc], mask[:, m * mc : (m + 1) * mc]
                )
            # ACT function tables: sigmoid and ln live in different table
            # sets (1283ns reload on switch), so run all sigmoids in the main
            # loop and one ln+accumulate pass at the end.
            LN_CHUNKS = min(8, T // G)
            GROUPS_PER_CHUNK = (T // G) // LN_CHUNKS
            CHUNK_COLS = G * 16 * GROUPS_PER_CHUNK
            tsum = constp.tile([P, LN_CHUNKS], F32)
            ln_scratch = constp.tile([P, CHUNK_COLS], F32)

            for g in range(T // G):
                emb = gatherp.tile([P, G * J * D], BF16, tag="emb")
                nc.gpsimd.indirect_dma_start(
                    out=emb[:],
                    out_offset=None,
                    in_=w[:],
                    in_offset=bass.IndirectOffsetOnAxis(
                        ap=idx_sb[:, g * G * J : (g + 1) * G * J], axis=0
                    ),
                )
                t0 = g * G
                # per-tile batch: emb_in for all G tiles in one op
                emb_in4 = smallp.tile([P, G * D], BF16, tag="embin")
                ei4 = emb_in4[:].rearrange("p (k d) -> p k d", d=D)
                nc.vector.tensor_tensor(
                    out=ei4,
                    in0=emb[:].rearrange("p (k j d) -> p k j d", j=J, d=D)[:, :, 0, :],
                    in1=mask_sb[:, t0 * D : (t0 + G) * D].rearrange(
                        "p (k d) -> p k d", d=D
                    ),
                    op=mybir.AluOpType.mult,
                )
                # fused custom DVE op: running sum of emb_j*emb_in across the
                # whole group stream; per-(tile,j) dots are then differences
                # at the 128-element segment boundaries.
                dot_scan = _get_dot_scan_op()
                scan4 = workp.tile([P, G * 16 * D], BF16, tag="prods")
                for k in range(G):
                    base = k * J * D
                    nc.vector._custom_dve(
                        dot_scan,
                        out=scan4[:, k * 16 * D : (k + 1) * 16 * D].rearrange(
                            "p (j d) -> p j d", d=D
                        ),
                        in0=emb[:, base + D : base + J * D].rearrange(
                            "p (j d) -> p j d", d=D
                        ),
                        in1=emb_in4[:, k * D : (k + 1) * D]
                        .unsqueeze(1)
                        .broadcast_to((P, 16, D)),
                    )
                S = G * 16
                bnd = scan4[:].rearrange("p (s d) -> p s d", d=D)[:, :, D - 1 : D]
                scores = smallp.tile([P, S], F32, tag="scores")
                # x[s] = cum[s] - cum[s-1] within each tile's scan; the scan
                # state resets per call, so tile-leading segments (s % 16 == 0)
                # take the raw boundary value instead (second op overwrites).
                nc.vector.tensor_tensor(
                    out=scores[:, 1:S].unsqueeze(2),
                    in0=bnd[:, 1:S, :],
                    in1=bnd[:, 0 : S - 1, :],
                    op=mybir.AluOpType.subtract,
                )
                nc.vector.tensor_copy(
                    scores[:, 0:S:16].unsqueeze(2), bnd[:, 0:S:16, :]
                )
                if g % GROUPS_PER_CHUNK == 0:
                    sig_c = smallp.tile([P, CHUNK_COLS], F32, tag="sigc")
                gc = g % GROUPS_PER_CHUNK
                nc.scalar.activation(
                    sig_c[:, gc * G * 16 : (gc + 1) * G * 16],
                    scores[:],
                    mybir.ActivationFunctionType.Sigmoid,
                    scale=-1.0,
                )
                # ln(sigmoid(-x)) = -softplus(x); accumulate per chunk,
                # interleaved so only the last chunk sits on the tail.
                if (g + 1) % GROUPS_PER_CHUNK == 0:
                    c = (g + 1) // GROUPS_PER_CHUNK - 1
                    nc.scalar.activation(
                        ln_scratch[:],
                        sig_c[:],
                        mybir.ActivationFunctionType.Ln,
                        accum_out=tsum[:, c : c + 1],
                    )

            total = constp.tile([P, 1], F32)
            nc.vector.tensor_reduce(
                total[:], tsum[:], axis=mybir.AxisListType.X, op=mybir.AluOpType.add
            )
            nc.sync.dma_start(out[:], total[:])
    nc.compile()
    return nc


def _get_nc():
    if "nc" not in _CACHE:
        _CACHE["nc"] = _build_nc()
    return _CACHE["nc"]


def kernel(target, context, neg_idx, dropout_mask, W_target, W_context):
    global LAST_RESULT
    nc = _get_nc()

    target = np.asarray(target).astype(np.int32, copy=False)
    context = np.asarray(context).astype(np.int32, copy=False)
    neg_idx = np.asarray(neg_idx).astype(np.int32, copy=False)
    dropout_mask = np.asarray(dropout_mask, dtype=np.float32)
    W_target = np.asarray(W_target, dtype=np.float32)
    W_context = np.asarray(W_context, dtype=np.float32)

    w_cat = np.ascontiguousarray(
        np.concatenate([W_target, W_context], axis=0).astype(NPBF16)
    )
    idx_cat = np.empty((B, J), np.int32)
    idx_cat[:, 0] = target
    idx_cat[:, 1] = context + V
    idx_cat[:, 2:] = neg_idx + V
    mask_bf = dropout_mask.astype(NPBF16)

    in_maps = []
    for c in range(NCORES):
        sl = slice(c * BLOC, (c + 1) * BLOC)
        idxs = np.ascontiguousarray(
            idx_cat[sl].reshape(T, P, J).transpose(1, 0, 2).reshape(P, T * J)
        )
        maskr = np.ascontiguousarray(
            mask_bf[sl].reshape(T, P, D).transpose(1, 0, 2).reshape(P, T * D)
        )
        in_maps.append({"w_cat": w_cat, "idx": idxs, "maskr": maskr})

    trace = bool(int(os.environ.get("KERNEL_TRACE", "0")))
    res = bass_utils.run_bass_kernel_spmd(
        nc, in_maps, core_ids=list(range(NCORES)), trace=trace
    )
    LAST_RESULT = res

    tot = 0.0
    for r in res.results:
        tot += float(r["out"].astype(np.float64).sum())
    # device accumulated sum of ln(sigmoid(-x)) = -sum of softplus(x)
    loss = -tot / B
    return np.asarray(np.float32(loss))
